# revision 5
# baseline (speedup 1.0000x reference)
"""Trainium2 Bass kernel for nn_MultiHeadAttention_30846455119878.

8-core strategy:
  - Attention phase is head-sharded: core m owns heads {2m, 2m+1}. Every core
    computes q/k/v projections for its 2 heads over all B*T tokens, then causal
    softmax attention per (batch, head).
  - The output projection contracts over ALL heads, so instead of an expensive
    AllReduce of [B,T,C] partials, each batch's attention output features
    ([128 feats x T]) are exchanged with a small AllToAll that re-shards from
    heads -> tokens. Each core then computes the full output projection for its
    1/8 token slice (contraction over all 1024 features) plus bias, locally.
  - Host side: x is passed pre-transposed as x^T [C, B*T] in bf16 (fp32 can't
    DMA-transpose on TRN2); all matmuls run bf16 x bf16 -> fp32 PSUM.

v2 restructure (softwar-pipelined phases). Profiling v1 (468us) showed:
  - attention inner loop is ScalarE-bound (exp streams ~29us/batch + ~175ns
    fixed cost per ACTIVATE), PE half-idle there;
  - projection phases are PE-bound with ScalarE idle;
  - staging DMAs on the scalar queue stole ~53us from exp;
  - 4x ~7.4us PE gaps at batch boundaries + 33us exposed tail (last AllToAll)
    + HAM/power throttling (avg clock limit 73%) from the idle gaps.
Changes:
  - proj(b+1) chunks interleaved into attention(b) per query-chunk -> PE and
    ScalarE overlap across batches, PE stays dense (HAM warm).
  - Both heads' scores land in one 2-bank PSUM tile [128, 2, 512]; ONE exp
    ACTIVATE + ONE mask multiply per key-block covers both heads.
  - Staging/rcv/out DMAs on the gpsimd queue (was scalar); den reshape DMAs on
    the vector queue (was sync, blocked xt prefetch); scalar queue = exp only.
  - rb normalize multiply reads rb_ps straight from PSUM (drops 32 ScalarE
    copies).
  - outproj(b-1) emitted after cc(b): overlaps attention(b+1)/the last cc.
    gpsimd queue order keeps stage(b) ahead of rcv(b-1) to avoid head-of-line
    deadlocks (rcv waits on an earlier cc sem only).
  - den buffers in bf16 (halves the reshape DMA).

Queue map: sync=xt+weights loads; scalar=exp; vector=DVE+den DMAs;
gpsimd=staging/cc/rcv/out/wo/bo; tensor=matmul only.

Perf notes from v1 kept: batched 128-lane reciprocal via SBUF reshape DMA;
diag-block column skip (c0); v computed directly in [s,d] layout; bf16
AllToAll + tile_position row-packing verified on HW; reciprocal_approx_fast /
gpsimd partition_broadcast BROKEN in this runtime; XBAR dma_start_transpose
shears with strided src; max 1 sync-wait per instruction -> bacc.Bacc.
"""

import sys

if "/opt/trn_rl_repo" not in sys.path:
    sys.path.insert(0, "/opt/trn_rl_repo")

import numpy as np
import ml_dtypes

import concourse.bass as bass
import concourse.tile as tile
from concourse import bacc, mybir
from concourse.bass_utils import run_bass_kernel_spmd
from concourse.tile_rust import add_dep_helper

BF16 = ml_dtypes.bfloat16

# Full problem dims
B_FULL, T_FULL, C_FULL, H_FULL, D_HEAD = 4, 2048, 1024, 16, 64
N_CORES = 8
HPC = H_FULL // N_CORES  # heads per core = 2
F = HPC * D_HEAD         # per-core attention feature rows = 128
TCH = 512                # query-chunk (free dim of score matmuls)
D = D_HEAD


def build_nc(B=B_FULL, T=T_FULL, C=C_FULL):
    """Build the SPMD Bass graph (same graph on all 8 cores)."""
    dt = mybir.dt
    CK = C // 128        # contraction chunks for projections
    NTC = T // TCH       # query chunks per sequence
    NSB = T // 128       # key blocks per sequence
    SBB = TCH // 128     # key blocks that overlap one query chunk diagonal = 4
    TS = T // N_CORES    # token shard per (batch, core) = 256
    CO = H_FULL * D_HEAD  # output feature dim (Wo cols) = 1024
    TT = 128             # token tile for output projection
    SLOTS = NTC * HPC    # denominator slots per batch = 8
    scale = float(1.0 / np.sqrt(C))

    nc = bacc.Bacc()
    xt_d = nc.declare_dram_parameter("xt", [128, CK, B * T], dt.bfloat16, isOutput=False)
    wq_d = nc.declare_dram_parameter("wq", [128, CK, F], dt.bfloat16, isOutput=False)
    wk_d = nc.declare_dram_parameter("wk", [128, CK, F], dt.bfloat16, isOutput=False)
    wv_d = nc.declare_dram_parameter("wv", [128, CK, F], dt.bfloat16, isOutput=False)
    wo_d = nc.declare_dram_parameter("wo", [128, N_CORES, CO], dt.bfloat16, isOutput=False)
    bo_d = nc.declare_dram_parameter("bo", [1, CO], dt.bfloat16, isOutput=False)
    mask_d = nc.declare_dram_parameter("mask", [128, SBB, HPC, TCH], dt.bfloat16, isOutput=False)
    out_d = nc.declare_dram_parameter("out", [B, TS, CO], dt.float32, isOutput=True)

    cc_in = [nc.dram_tensor(f"cc_in{b}", [N_CORES, F, TS], dt.bfloat16) for b in range(B)]
    cc_out = [nc.dram_tensor(f"cc_out{b}", [N_CORES, F, TS], dt.bfloat16) for b in range(B)]
    rg = [list(range(N_CORES))]

    with tile.TileContext(nc) as tc:
        from contextlib import ExitStack

        with ExitStack() as ctx:
            wpool = ctx.enter_context(tc.tile_pool(name="w", bufs=1))
            xpool = ctx.enter_context(tc.tile_pool(name="xt", bufs=3))
            qkpool = ctx.enter_context(tc.tile_pool(name="qk", bufs=2))
            v1pool = ctx.enter_context(tc.tile_pool(name="v1", bufs=2))
            epool = ctx.enter_context(tc.tile_pool(name="exp", bufs=6))
            apool = ctx.enter_context(tc.tile_pool(name="attn", bufs=4))
            recpool = ctx.enter_context(tc.tile_pool(name="rec", bufs=2))
            aupool = ctx.enter_context(tc.tile_pool(name="attu", bufs=2))
            denpool = ctx.enter_context(tc.tile_pool(name="den", bufs=2))
            rcvpool = ctx.enter_context(tc.tile_pool(name="rcv", bufs=2))
            outpool = ctx.enter_context(tc.tile_pool(name="osb", bufs=2))
            psA = ctx.enter_context(tc.tile_pool(name="psA", bufs=2, space="PSUM"))
            psS = ctx.enter_context(tc.tile_pool(name="psS", bufs=2, space="PSUM"))
            psB = ctx.enter_context(tc.tile_pool(name="psB", bufs=2, space="PSUM"))

            # resident constants; xt chunk DMAs are issued by proj chunks on
            # the same sync queue.  wo/bo (2MB, needed only at outproj) go on
            # the gpsimd queue so they don't delay the first projections.
            wq_sb = wpool.tile([128, CK, F], dt.bfloat16, tag="wq")
            wk_sb = wpool.tile([128, CK, F], dt.bfloat16, tag="wk")
            wv_sb = wpool.tile([128, CK, F], dt.bfloat16, tag="wv")
            wo_sb = wpool.tile([128, N_CORES, CO], dt.bfloat16, tag="wo")
            bo_sb = wpool.tile([1, CO], dt.bfloat16, tag="bo")
            mask_sb = wpool.tile([128, SBB, HPC, TCH], dt.bfloat16, tag="mask")
            ones_sb = wpool.tile([D + 1, 128], dt.bfloat16, tag="ones")
            nc.sync.dma_start(out=wq_sb, in_=wq_d[:, :, :])
            nc.sync.dma_start(out=wk_sb, in_=wk_d[:, :, :])
            nc.scalar.dma_start(out=wv_sb, in_=wv_d[:, :, :])
            nc.scalar.dma_start(out=mask_sb, in_=mask_d[:, :, :, :])
            nc.gpsimd.dma_start(out=wo_sb, in_=wo_d[:, :, :])
            nc.gpsimd.dma_start(out=bo_sb, in_=bo_d[:, :])
            nc.vector.memset(ones_sb, 1.0)

            # -------- per-batch projection state and one-chunk emitter ------
            def new_proj_state(b):
                qT = qkpool.tile([F, T], dt.bfloat16, tag="qT", name=f"qT_{b}")
                kT = qkpool.tile([F, T], dt.bfloat16, tag="kT", name=f"kT_{b}")
                v1 = v1pool.tile([128, NSB, HPC, 80], dt.bfloat16, tag="v1", name=f"v1_{b}")
                nc.vector.memset(v1[:, :, :, D:D + 1], 1.0)
                return {"qT": qT, "kT": kT, "v1": v1}

            def emit_proj_chunk(b, tcb, st):
                g0 = b * T + tcb * TCH
                xt_sb = xpool.tile([128, CK, TCH], dt.bfloat16, tag="xt",
                                   name=f"xt_{b}_{tcb}")
                nc.sync.dma_start(out=xt_sb, in_=xt_d[:, :, g0:g0 + TCH])
                for w_sb, dstT in ((wq_sb, st["qT"]), (wk_sb, st["kT"])):
                    ps = psA.tile([128, TCH], dt.float32, tag="mm")
                    for o in range(CK):
                        nc.tensor.matmul(
                            ps, lhsT=w_sb[:, o, :], rhs=xt_sb[:, o, :],
                            start=(o == 0), stop=(o == CK - 1),
                        )
                    nc.vector.tensor_copy(
                        out=dstT[:, tcb * TCH:(tcb + 1) * TCH], in_=ps
                    )
                # v directly in [s, d] layout: v[s, f] = sum_c x[s, c] Wv[c, f]
                for ssub in range(SBB):
                    vps_full = psA.tile([128, TCH], dt.float32, tag="mm",
                                        name=f"vps_{b}_{tcb}_{ssub}")
                    vps = vps_full[:, 0:F]
                    for o in range(CK):
                        nc.tensor.matmul(
                            vps,
                            lhsT=xt_sb[:, o, ssub * 128:(ssub + 1) * 128],
                            rhs=wv_sb[:, o, :],
                            start=(o == 0), stop=(o == CK - 1),
                        )
                    stx = tcb * SBB + ssub
                    for h in range(HPC):
                        nc.vector.tensor_copy(
                            out=st["v1"][:, stx, h, 0:D], in_=vps[:, h * D:(h + 1) * D]
                        )

            # -------- output projection emitter (per batch) -----------------
            def emit_outproj(b, eng=None):
                eng = eng if eng is not None else nc.gpsimd
                rcv = rcvpool.tile([128, N_CORES, TS], dt.bfloat16, tag="rcv",
                                   name=f"rcv_{b}")
                rcv_rd = eng.dma_start(
                    out=rcv, in_=cc_out[b][:, :, :].rearrange("j p t -> p j t")
                )
                add_dep_helper(rcv_rd.ins, cc_insts[b], sync=True, reason="cc_out RAW")
                for tt in range(TS // TT):
                    for c2 in range(CO // 512):
                        ps = psA.tile([128, TCH], dt.float32, tag="mm",
                                      name=f"ops_{b}_{tt}_{c2}")
                        for j in range(N_CORES):
                            nc.tensor.matmul(
                                ps[0:TT, 0:512],
                                lhsT=rcv[:, j, tt * TT:(tt + 1) * TT],
                                rhs=wo_sb[:, j, c2 * 512:(c2 + 1) * 512],
                                start=(j == 0), stop=False,
                            )
                        nc.tensor.matmul(
                            ps[0:TT, 0:512],
                            lhsT=ones_sb[0:1, 0:TT],
                            rhs=bo_sb[0:1, c2 * 512:(c2 + 1) * 512],
                            start=False, stop=True,
                        )
                        osb = outpool.tile([TT, 512], dt.float32, tag="osb",
                                           name=f"osb_{b}_{tt}_{c2}")
                        nc.vector.tensor_copy(out=osb, in_=ps[0:TT, 0:512])
                        eng.dma_start(
                            out=out_d[b, tt * TT:(tt + 1) * TT, c2 * 512:(c2 + 1) * 512],
                            in_=osb,
                        )

            # ---------------- main pipelined batch loop ---------------------
            cc_insts = []
            st = [None] * B
            st[0] = new_proj_state(0)
            for tcb in range(NTC):
                emit_proj_chunk(0, tcb, st[0])

            for b in range(B):
                # output projection of the PREVIOUS batch: its collective
                # completes early in this batch's attention; the matmuls fill
                # PE slack.  For b=3 the rcv/out DMAs ride the scalar queue
                # (idle after the last exp) so outproj(2) can also overlap
                # cc(3), whose trigger blocks the gpsimd queue to completion.
                if b >= 1:
                    emit_outproj(b - 1, eng=nc.scalar if b == B - 1 else nc.gpsimd)
                if b + 1 < B:
                    st[b + 1] = new_proj_state(b + 1)
                qT, kT, v1 = st[b]["qT"], st[b]["kT"], st[b]["v1"]

                # ---- causal attention, both heads in one exp/mask op
                attn_h = [apool.tile([D, T], dt.bfloat16, tag="attn",
                                     name=f"attn_{b}_{hh}") for hh in range(HPC)]
                att_un = aupool.tile([D, SLOTS, TCH], dt.bfloat16, tag="attu",
                                     name=f"attu_{b}")
                den_b = denpool.tile([1, SLOTS * TCH], dt.bfloat16, tag="den",
                                     name=f"den_{b}")
                for tcb in range(NTC):
                    att_ps = [psB.tile([D + 1, TCH], dt.float32, tag="att",
                                       name=f"attps_{b}_{tcb}_{hh}")
                              for hh in range(HPC)]
                    nsb = SBB * (tcb + 1)
                    for sb in range(nsb):
                        j0 = sb - SBB * tcb
                        # columns t < j0*128 of this (key-block, query-chunk)
                        # pair are fully causal-masked -> skipped everywhere
                        c0 = j0 * 128 if j0 > 0 else 0
                        sps = psS.tile([128, HPC, TCH], dt.float32, tag="sps",
                                       name=f"sps_{b}_{tcb}_{sb}")
                        for h in range(HPC):
                            nc.tensor.matmul(
                                sps[:, h, c0:TCH],
                                lhsT=kT[h * D:(h + 1) * D, sb * 128:(sb + 1) * 128],
                                rhs=qT[h * D:(h + 1) * D, tcb * TCH + c0:(tcb + 1) * TCH],
                                start=True, stop=True,
                                tile_position=(h * D, 0),
                            )
                        et = epool.tile([128, HPC, TCH], dt.bfloat16, tag="exp",
                                        name=f"et_{b}_{tcb}_{sb}")
                        nc.scalar.activation(
                            out=et[:, :, c0:TCH], in_=sps[:, :, c0:TCH],
                            func=mybir.ActivationFunctionType.Exp, scale=scale,
                        )
                        if j0 >= 0:
                            nc.vector.tensor_mul(
                                et[:, :, c0:TCH], et[:, :, c0:TCH],
                                mask_sb[:, j0, :, c0:TCH],
                            )
                        for h in range(HPC):
                            nc.tensor.matmul(
                                att_ps[h][:, c0:TCH],
                                lhsT=v1[:, sb, h, 0:D + 1], rhs=et[:, h, c0:TCH],
                                start=(sb == 0), stop=(sb == nsb - 1),
                            )
                    for h in range(HPC):
                        slot = tcb * HPC + h
                        # copy unnormalized attention + denominator out of PSUM
                        nc.vector.tensor_copy(out=att_un[:, slot, :], in_=att_ps[h][0:D, :])
                        nc.vector.tensor_copy(
                            out=den_b[0:1, slot * TCH:(slot + 1) * TCH],
                            in_=att_ps[h][D:D + 1, :],
                        )
                    # overlap next batch's projections with this attention
                    if b + 1 < B:
                        emit_proj_chunk(b + 1, tcb, st[b + 1])

                # batch-reciprocal all denominators across 128 lanes
                den_t = recpool.tile([128, SLOTS * TCH // 128], dt.bfloat16,
                                     tag="dent", name=f"dent_{b}")
                nc.sync.dma_start(out=den_t, in_=den_b[0:1, :])
                rec_t = recpool.tile([128, SLOTS * TCH // 128], dt.bfloat16,
                                     tag="rect", name=f"rect_{b}")
                with nc.allow_low_precision(reason="bf16 softmax denom recip is plenty at rel-err 2e-2"):
                    nc.vector.reciprocal(out=rec_t, in_=den_t)
                rec_all = recpool.tile([1, SLOTS * TCH], dt.bfloat16,
                                       tag="recall", name=f"recall_{b}")
                nc.sync.dma_start(out=rec_all, in_=rec_t)
                # broadcast 1/den over the 64 feature rows and normalize
                for tcb in range(NTC):
                    for h in range(HPC):
                        slot = tcb * HPC + h
                        rb_ps = psA.tile([D, TCH], dt.float32, tag="mm",
                                         name=f"rb_{b}_{slot}")
                        nc.tensor.matmul(
                            rb_ps, lhsT=ones_sb[0:1, 0:D],
                            rhs=rec_all[0:1, slot * TCH:(slot + 1) * TCH],
                            start=True, stop=True,
                        )
                        nc.vector.tensor_mul(
                            attn_h[h][:, tcb * TCH:(tcb + 1) * TCH],
                            att_un[:, slot, :], rb_ps,
                        )
                stg_insts = []
                for h in range(HPC):
                    for j in range(N_CORES):
                        stg_insts.append(nc.gpsimd.dma_start(
                            out=cc_in[b][j, h * D:(h + 1) * D, :],
                            in_=attn_h[h][:, j * TS:(j + 1) * TS],
                        ).ins)
                cc = nc.gpsimd.collective_compute(
                    "AllToAll", mybir.AluOpType.bypass, replica_groups=rg,
                    ins=[cc_in[b].ap().opt()], outs=[cc_out[b].ap().opt()],
                )
                for s in stg_insts:
                    add_dep_helper(cc.ins, s, sync=True, reason="cc_in RAW")
                cc_insts.append(cc.ins)

            emit_outproj(B - 1)

    nc.finalize()
    return nc


def prep_inputs(x, Wq, Wk, Wv, Wo, bo):
    """Host-side shard/layout prep. Returns in_maps for the 8 cores."""
    B, T, C = x.shape
    CK = C // 128
    SBB = TCH // 128

    x = np.asarray(x, dtype=np.float32)
    xt = np.ascontiguousarray(x.reshape(B * T, C).T.astype(BF16))  # [C, B*T]
    xt = np.ascontiguousarray(xt.reshape(CK, 128, B * T).transpose(1, 0, 2))

    CO = Wo.shape[1]
    wo_h = np.ascontiguousarray(
        np.asarray(Wo, np.float32).astype(BF16).reshape(N_CORES, 128, CO).transpose(1, 0, 2)
    )
    bo_h = np.asarray(bo, np.float32).astype(BF16).reshape(1, CO)

    p = np.arange(128)[:, None, None]
    j = np.arange(SBB)[None, :, None]
    t = np.arange(TCH)[None, None, :]
    mask_h = (t >= p + j * 128).astype(BF16)          # [128, SBB, TCH]
    mask_h = np.ascontiguousarray(
        np.broadcast_to(mask_h[:, :, None, :], (128, SBB, HPC, TCH))
    )

    in_maps = []
    for m in range(N_CORES):
        maps = {"xt": xt, "wo": wo_h, "bo": bo_h, "mask": mask_h}
        for name, W in (("wq", Wq), ("wk", Wk), ("wv", Wv)):
            Ws = np.concatenate(
                [np.asarray(W[HPC * m + i], np.float32) for i in range(HPC)], axis=1
            )  # [C, F]
            maps[name] = np.ascontiguousarray(
                Ws.astype(BF16).reshape(CK, 128, F).transpose(1, 0, 2)
            )
        in_maps.append(maps)
    return in_maps


_NC_CACHE = {}


def _get_nc(B, T, C):
    key = (B, T, C)
    if key not in _NC_CACHE:
        _NC_CACHE[key] = build_nc(B, T, C)
    return _NC_CACHE[key]


def kernel(x, Wq, Wk, Wv, Wo, bo, _trace=False):
    x = np.asarray(x)
    B, T, C = x.shape
    nc = _get_nc(B, T, C)
    in_maps = prep_inputs(x, Wq, Wk, Wv, Wo, bo)
    res = run_bass_kernel_spmd(
        nc, in_maps, core_ids=list(range(N_CORES)), trace=_trace
    )
    TS = T // N_CORES
    CO = np.asarray(Wo).shape[1]
    out = np.empty((B, T, CO), dtype=np.float32)
    for m in range(N_CORES):
        out[:, m * TS:(m + 1) * TS, :] = res.results[m]["out"]
    if _trace:
        kernel.last_result = res
    return out


# revision 6
# speedup vs baseline: 1.0713x; 1.0713x over previous
"""Trainium2 Bass kernel for nn_MultiHeadAttention_30846455119878.

8-core strategy:
  - Attention phase is head-sharded: core m owns heads {2m, 2m+1}. Every core
    computes q/k/v projections for its 2 heads over all B*T tokens, then causal
    softmax attention per (batch, head).
  - The output projection contracts over ALL heads, so instead of an expensive
    AllReduce of [B,T,C] partials, each batch's attention output features
    ([128 feats x T]) are exchanged with a small AllToAll that re-shards from
    heads -> tokens. Each core then computes the full output projection for its
    1/8 token slice (contraction over all 1024 features) plus bias, locally.
  - Host side: x is passed pre-transposed as x^T [C, B*T] in bf16 (fp32 can't
    DMA-transpose on TRN2); all matmuls run bf16 x bf16 -> fp32 PSUM.

v2 restructure (softwar-pipelined phases). Profiling v1 (468us) showed:
  - attention inner loop is ScalarE-bound (exp streams ~29us/batch + ~175ns
    fixed cost per ACTIVATE), PE half-idle there;
  - projection phases are PE-bound with ScalarE idle;
  - staging DMAs on the scalar queue stole ~53us from exp;
  - 4x ~7.4us PE gaps at batch boundaries + 33us exposed tail (last AllToAll)
    + HAM/power throttling (avg clock limit 73%) from the idle gaps.
Changes:
  - proj(b+1) chunks interleaved into attention(b) per query-chunk -> PE and
    ScalarE overlap across batches, PE stays dense (HAM warm).
  - Both heads' scores land in one 2-bank PSUM tile [128, 2, 512]; ONE exp
    ACTIVATE + ONE mask multiply per key-block covers both heads.
  - Staging/rcv/out DMAs on the gpsimd queue (was scalar); den reshape DMAs on
    the vector queue (was sync, blocked xt prefetch); scalar queue = exp only.
  - rb normalize multiply reads rb_ps straight from PSUM (drops 32 ScalarE
    copies).
  - outproj(b-1) emitted after cc(b): overlaps attention(b+1)/the last cc.
    gpsimd queue order keeps stage(b) ahead of rcv(b-1) to avoid head-of-line
    deadlocks (rcv waits on an earlier cc sem only).
  - den buffers in bf16 (halves the reshape DMA).

Queue map: sync=xt+weights loads; scalar=exp; vector=DVE+den DMAs;
gpsimd=staging/cc/rcv/out/wo/bo; tensor=matmul only.

Perf notes from v1 kept: batched 128-lane reciprocal via SBUF reshape DMA;
diag-block column skip (c0); v computed directly in [s,d] layout; bf16
AllToAll + tile_position row-packing verified on HW; reciprocal_approx_fast /
gpsimd partition_broadcast BROKEN in this runtime; XBAR dma_start_transpose
shears with strided src; max 1 sync-wait per instruction -> bacc.Bacc.
"""

import sys

if "/opt/trn_rl_repo" not in sys.path:
    sys.path.insert(0, "/opt/trn_rl_repo")

import numpy as np
import ml_dtypes

import concourse.bass as bass
import concourse.tile as tile
from concourse import bacc, mybir
from concourse.bass_utils import run_bass_kernel_spmd
from concourse.tile_rust import add_dep_helper

BF16 = ml_dtypes.bfloat16

# Full problem dims
B_FULL, T_FULL, C_FULL, H_FULL, D_HEAD = 4, 2048, 1024, 16, 64
N_CORES = 8
HPC = H_FULL // N_CORES  # heads per core = 2
F = HPC * D_HEAD         # per-core attention feature rows = 128
TCH = 512                # query-chunk (free dim of score matmuls)
D = D_HEAD


def build_nc(B=B_FULL, T=T_FULL, C=C_FULL):
    """Build the SPMD Bass graph (same graph on all 8 cores)."""
    dt = mybir.dt
    CK = C // 128        # contraction chunks for projections
    NTC = T // TCH       # query chunks per sequence
    NSB = T // 128       # key blocks per sequence
    SBB = TCH // 128     # key blocks that overlap one query chunk diagonal = 4
    TS = T // N_CORES    # token shard per (batch, core) = 256
    CO = H_FULL * D_HEAD  # output feature dim (Wo cols) = 1024
    TT = 128             # token tile for output projection
    SLOTS = NTC * HPC    # denominator slots per batch = 8
    scale = float(1.0 / np.sqrt(C))

    nc = bacc.Bacc()
    xt_d = nc.declare_dram_parameter("xt", [128, CK, B * T], dt.bfloat16, isOutput=False)
    wq_d = nc.declare_dram_parameter("wq", [128, CK, F], dt.bfloat16, isOutput=False)
    wk_d = nc.declare_dram_parameter("wk", [128, CK, F], dt.bfloat16, isOutput=False)
    wv_d = nc.declare_dram_parameter("wv", [128, CK, F], dt.bfloat16, isOutput=False)
    wo_d = nc.declare_dram_parameter("wo", [128, N_CORES, CO], dt.bfloat16, isOutput=False)
    bo_d = nc.declare_dram_parameter("bo", [1, CO], dt.bfloat16, isOutput=False)
    mask_d = nc.declare_dram_parameter("mask", [128, SBB, TCH], dt.bfloat16, isOutput=False)
    out_d = nc.declare_dram_parameter("out", [B, TS, CO], dt.float32, isOutput=True)

    cc_in = [nc.dram_tensor(f"cc_in{b}", [N_CORES, F, TS], dt.bfloat16) for b in range(B)]
    cc_out = [nc.dram_tensor(f"cc_out{b}", [N_CORES, F, TS], dt.bfloat16) for b in range(B)]
    rg = [list(range(N_CORES))]

    with tile.TileContext(nc) as tc:
        from contextlib import ExitStack

        with ExitStack() as ctx:
            wpool = ctx.enter_context(tc.tile_pool(name="w", bufs=1))
            xpool = ctx.enter_context(tc.tile_pool(name="xt", bufs=3))
            qkpool = ctx.enter_context(tc.tile_pool(name="qk", bufs=2))
            v1pool = ctx.enter_context(tc.tile_pool(name="v1", bufs=2))
            epool = ctx.enter_context(tc.tile_pool(name="exp", bufs=6))
            apool = ctx.enter_context(tc.tile_pool(name="attn", bufs=4))
            recpool = ctx.enter_context(tc.tile_pool(name="rec", bufs=2))
            aupool = ctx.enter_context(tc.tile_pool(name="attu", bufs=2))
            denpool = ctx.enter_context(tc.tile_pool(name="den", bufs=2))
            rcvpool = ctx.enter_context(tc.tile_pool(name="rcv", bufs=2))
            outpool = ctx.enter_context(tc.tile_pool(name="osb", bufs=2))
            psA = ctx.enter_context(tc.tile_pool(name="psA", bufs=2, space="PSUM"))
            psS = ctx.enter_context(tc.tile_pool(name="psS", bufs=2, space="PSUM"))
            psB = ctx.enter_context(tc.tile_pool(name="psB", bufs=2, space="PSUM"))

            # resident constants; xt chunk DMAs are issued by proj chunks on
            # the same sync queue.  wo/bo (2MB, needed only at outproj) go on
            # the gpsimd queue so they don't delay the first projections.
            wq_sb = wpool.tile([128, CK, F], dt.bfloat16, tag="wq")
            wk_sb = wpool.tile([128, CK, F], dt.bfloat16, tag="wk")
            wv_sb = wpool.tile([128, CK, F], dt.bfloat16, tag="wv")
            wo_sb = wpool.tile([128, N_CORES, CO], dt.bfloat16, tag="wo")
            bo_sb = wpool.tile([1, CO], dt.bfloat16, tag="bo")
            mask_sb = wpool.tile([128, SBB, TCH], dt.bfloat16, tag="mask")
            ones_sb = wpool.tile([D + 1, 128], dt.bfloat16, tag="ones")
            nc.sync.dma_start(out=wq_sb, in_=wq_d[:, :, :])
            nc.sync.dma_start(out=wk_sb, in_=wk_d[:, :, :])
            nc.scalar.dma_start(out=wv_sb, in_=wv_d[:, :, :])
            nc.scalar.dma_start(out=mask_sb, in_=mask_d[:, :, :])
            nc.vector.memset(ones_sb, 1.0)

            # -------- per-batch projection state and one-chunk emitter ------
            def new_proj_state(b):
                qT = qkpool.tile([F, T], dt.bfloat16, tag="qT", name=f"qT_{b}")
                kT = qkpool.tile([F, T], dt.bfloat16, tag="kT", name=f"kT_{b}")
                v1 = v1pool.tile([128, NSB, HPC, 80], dt.bfloat16, tag="v1", name=f"v1_{b}")
                nc.vector.memset(v1[:, :, :, D:D + 1], 1.0)
                return {"qT": qT, "kT": kT, "v1": v1}

            def emit_proj_chunk(b, tcb, st):
                g0 = b * T + tcb * TCH
                xt_sb = xpool.tile([128, CK, TCH], dt.bfloat16, tag="xt",
                                   name=f"xt_{b}_{tcb}")
                nc.sync.dma_start(out=xt_sb, in_=xt_d[:, :, g0:g0 + TCH])
                for w_sb, dstT in ((wq_sb, st["qT"]), (wk_sb, st["kT"])):
                    ps = psA.tile([128, TCH], dt.float32, tag="mm")
                    for o in range(CK):
                        nc.tensor.matmul(
                            ps, lhsT=w_sb[:, o, :], rhs=xt_sb[:, o, :],
                            start=(o == 0), stop=(o == CK - 1),
                        )
                    nc.vector.tensor_copy(
                        out=dstT[:, tcb * TCH:(tcb + 1) * TCH], in_=ps
                    )
                # v directly in [s, d] layout: v[s, f] = sum_c x[s, c] Wv[c, f]
                for ssub in range(SBB):
                    vps_full = psA.tile([128, TCH], dt.float32, tag="mm",
                                        name=f"vps_{b}_{tcb}_{ssub}")
                    vps = vps_full[:, 0:F]
                    for o in range(CK):
                        nc.tensor.matmul(
                            vps,
                            lhsT=xt_sb[:, o, ssub * 128:(ssub + 1) * 128],
                            rhs=wv_sb[:, o, :],
                            start=(o == 0), stop=(o == CK - 1),
                        )
                    stx = tcb * SBB + ssub
                    for h in range(HPC):
                        nc.vector.tensor_copy(
                            out=st["v1"][:, stx, h, 0:D], in_=vps[:, h * D:(h + 1) * D]
                        )

            # -------- output projection emitter (per batch) -----------------
            def emit_outproj(b, eng=None):
                eng = eng if eng is not None else nc.gpsimd
                rcv = rcvpool.tile([128, N_CORES, TS], dt.bfloat16, tag="rcv",
                                   name=f"rcv_{b}")
                rcv_rd = eng.dma_start(
                    out=rcv, in_=cc_out[b][:, :, :].rearrange("j p t -> p j t")
                )
                add_dep_helper(rcv_rd.ins, cc_insts[b], sync=True, reason="cc_out RAW")
                for tt in range(TS // TT):
                    for c2 in range(CO // 512):
                        ps = psA.tile([128, TCH], dt.float32, tag="mm",
                                      name=f"ops_{b}_{tt}_{c2}")
                        for j in range(N_CORES):
                            nc.tensor.matmul(
                                ps[0:TT, 0:512],
                                lhsT=rcv[:, j, tt * TT:(tt + 1) * TT],
                                rhs=wo_sb[:, j, c2 * 512:(c2 + 1) * 512],
                                start=(j == 0), stop=False,
                            )
                        nc.tensor.matmul(
                            ps[0:TT, 0:512],
                            lhsT=ones_sb[0:1, 0:TT],
                            rhs=bo_sb[0:1, c2 * 512:(c2 + 1) * 512],
                            start=False, stop=True,
                        )
                        osb = outpool.tile([TT, 512], dt.float32, tag="osb",
                                           name=f"osb_{b}_{tt}_{c2}")
                        nc.vector.tensor_copy(out=osb, in_=ps[0:TT, 0:512])
                        eng.dma_start(
                            out=out_d[b, tt * TT:(tt + 1) * TT, c2 * 512:(c2 + 1) * 512],
                            in_=osb,
                        )

            # ---------------- main pipelined batch loop ---------------------
            cc_insts = []
            st = [None] * B
            st[0] = new_proj_state(0)
            for tcb in range(NTC):
                emit_proj_chunk(0, tcb, st[0])

            for b in range(B):
                if b + 1 < B:
                    st[b + 1] = new_proj_state(b + 1)
                qT, kT, v1 = st[b]["qT"], st[b]["kT"], st[b]["v1"]

                # ---- causal attention, both heads in one exp/mask op
                attn_h = [apool.tile([D, T], dt.bfloat16, tag="attn",
                                     name=f"attn_{b}_{hh}") for hh in range(HPC)]
                att_un = aupool.tile([D, SLOTS, TCH], dt.bfloat16, tag="attu",
                                     name=f"attu_{b}")
                den_b = denpool.tile([1, SLOTS * TCH], dt.bfloat16, tag="den",
                                     name=f"den_{b}")
                for tcb in range(NTC):
                    att_ps = [psB.tile([D + 1, TCH], dt.float32, tag="att",
                                       name=f"attps_{b}_{tcb}_{hh}")
                              for hh in range(HPC)]
                    nsb = SBB * (tcb + 1)
                    for sb in range(nsb):
                        j0 = sb - SBB * tcb
                        # columns t < j0*128 of this (key-block, query-chunk)
                        # pair are fully causal-masked -> skipped everywhere
                        c0 = j0 * 128 if j0 > 0 else 0
                        sps = psS.tile([128, HPC, TCH], dt.float32, tag="sps",
                                       name=f"sps_{b}_{tcb}_{sb}")
                        for h in range(HPC):
                            nc.tensor.matmul(
                                sps[:, h, c0:TCH],
                                lhsT=kT[h * D:(h + 1) * D, sb * 128:(sb + 1) * 128],
                                rhs=qT[h * D:(h + 1) * D, tcb * TCH + c0:(tcb + 1) * TCH],
                                start=True, stop=True,
                                tile_position=(h * D, 0),
                            )
                        et = epool.tile([128, HPC, TCH], dt.bfloat16, tag="exp",
                                        name=f"et_{b}_{tcb}_{sb}")
                        nc.scalar.activation(
                            out=et[:, :, c0:TCH], in_=sps[:, :, c0:TCH],
                            func=mybir.ActivationFunctionType.Exp, scale=scale,
                        )
                        if j0 >= 0:
                            for h in range(HPC):
                                nc.vector.tensor_mul(
                                    et[:, h, c0:TCH], et[:, h, c0:TCH],
                                    mask_sb[:, j0, c0:TCH],
                                )
                        for h in range(HPC):
                            nc.tensor.matmul(
                                att_ps[h][:, c0:TCH],
                                lhsT=v1[:, sb, h, 0:D + 1], rhs=et[:, h, c0:TCH],
                                start=(sb == 0), stop=(sb == nsb - 1),
                            )
                    for h in range(HPC):
                        slot = tcb * HPC + h
                        # copy unnormalized attention + denominator out of PSUM
                        nc.vector.tensor_copy(out=att_un[:, slot, :], in_=att_ps[h][0:D, :])
                        nc.vector.tensor_copy(
                            out=den_b[0:1, slot * TCH:(slot + 1) * TCH],
                            in_=att_ps[h][D:D + 1, :],
                        )
                    # overlap next batch's projections with this attention
                    if b + 1 < B:
                        emit_proj_chunk(b + 1, tcb, st[b + 1])

                # batch-reciprocal all denominators across 128 lanes
                den_t = recpool.tile([128, SLOTS * TCH // 128], dt.bfloat16,
                                     tag="dent", name=f"dent_{b}")
                nc.sync.dma_start(out=den_t, in_=den_b[0:1, :])
                rec_t = recpool.tile([128, SLOTS * TCH // 128], dt.bfloat16,
                                     tag="rect", name=f"rect_{b}")
                with nc.allow_low_precision(reason="bf16 softmax denom recip is plenty at rel-err 2e-2"):
                    nc.vector.reciprocal(out=rec_t, in_=den_t)
                rec_all = recpool.tile([1, SLOTS * TCH], dt.bfloat16,
                                       tag="recall", name=f"recall_{b}")
                nc.sync.dma_start(out=rec_all, in_=rec_t)
                # broadcast 1/den over the 64 feature rows and normalize
                for tcb in range(NTC):
                    for h in range(HPC):
                        slot = tcb * HPC + h
                        rb_ps = psA.tile([D, TCH], dt.float32, tag="mm",
                                         name=f"rb_{b}_{slot}")
                        nc.tensor.matmul(
                            rb_ps, lhsT=ones_sb[0:1, 0:D],
                            rhs=rec_all[0:1, slot * TCH:(slot + 1) * TCH],
                            start=True, stop=True,
                        )
                        nc.vector.tensor_mul(
                            attn_h[h][:, tcb * TCH:(tcb + 1) * TCH],
                            att_un[:, slot, :], rb_ps,
                        )
                if b == 0:
                    # wo/bo needed first at outproj(0) (~cc(0) done); loading
                    # them here keeps 2.5MB of HBM traffic out of the startup
                    # projection's critical path.
                    nc.gpsimd.dma_start(out=wo_sb, in_=wo_d[:, :, :])
                    nc.gpsimd.dma_start(out=bo_sb, in_=bo_d[:, :])
                stg_insts = []
                for h in range(HPC):
                    for j in range(N_CORES):
                        stg_insts.append(nc.gpsimd.dma_start(
                            out=cc_in[b][j, h * D:(h + 1) * D, :],
                            in_=attn_h[h][:, j * TS:(j + 1) * TS],
                        ).ins)
                cc = nc.gpsimd.collective_compute(
                    "AllToAll", mybir.AluOpType.bypass, replica_groups=rg,
                    ins=[cc_in[b].ap().opt()], outs=[cc_out[b].ap().opt()],
                )
                for s in stg_insts:
                    add_dep_helper(cc.ins, s, sync=True, reason="cc_in RAW")
                cc_insts.append(cc.ins)
                # output projection of the PREVIOUS batch: its collective is
                # long done; the matmuls overlap attention(b+1).  Emitted
                # after cc(b) so its psA allocations sit behind rb(b) in the
                # pool ring (emitting earlier head-of-line-blocks proj(b+2)'s
                # PSUM tiles on the not-yet-ready rcv).  For the last batch
                # the rcv/out DMAs ride the scalar queue (idle after the last
                # exp): the gpsimd queue is blocked until cc(3) completes,
                # scalar is not, so outproj(2) overlaps cc(3).
                if b >= 1:
                    emit_outproj(b - 1, eng=nc.scalar if b == B - 1 else nc.gpsimd)

            emit_outproj(B - 1)

    nc.finalize()
    return nc


def prep_inputs(x, Wq, Wk, Wv, Wo, bo):
    """Host-side shard/layout prep. Returns in_maps for the 8 cores."""
    B, T, C = x.shape
    CK = C // 128
    SBB = TCH // 128

    x = np.asarray(x, dtype=np.float32)
    xt = np.ascontiguousarray(x.reshape(B * T, C).T.astype(BF16))  # [C, B*T]
    xt = np.ascontiguousarray(xt.reshape(CK, 128, B * T).transpose(1, 0, 2))

    CO = Wo.shape[1]
    wo_h = np.ascontiguousarray(
        np.asarray(Wo, np.float32).astype(BF16).reshape(N_CORES, 128, CO).transpose(1, 0, 2)
    )
    bo_h = np.asarray(bo, np.float32).astype(BF16).reshape(1, CO)

    p = np.arange(128)[:, None, None]
    j = np.arange(SBB)[None, :, None]
    t = np.arange(TCH)[None, None, :]
    mask_h = (t >= p + j * 128).astype(BF16)          # [128, SBB, TCH]

    in_maps = []
    for m in range(N_CORES):
        maps = {"xt": xt, "wo": wo_h, "bo": bo_h, "mask": mask_h}
        for name, W in (("wq", Wq), ("wk", Wk), ("wv", Wv)):
            Ws = np.concatenate(
                [np.asarray(W[HPC * m + i], np.float32) for i in range(HPC)], axis=1
            )  # [C, F]
            maps[name] = np.ascontiguousarray(
                Ws.astype(BF16).reshape(CK, 128, F).transpose(1, 0, 2)
            )
        in_maps.append(maps)
    return in_maps


_NC_CACHE = {}


def _get_nc(B, T, C):
    key = (B, T, C)
    if key not in _NC_CACHE:
        _NC_CACHE[key] = build_nc(B, T, C)
    return _NC_CACHE[key]


def kernel(x, Wq, Wk, Wv, Wo, bo, _trace=False):
    x = np.asarray(x)
    B, T, C = x.shape
    nc = _get_nc(B, T, C)
    in_maps = prep_inputs(x, Wq, Wk, Wv, Wo, bo)
    res = run_bass_kernel_spmd(
        nc, in_maps, core_ids=list(range(N_CORES)), trace=_trace
    )
    TS = T // N_CORES
    CO = np.asarray(Wo).shape[1]
    out = np.empty((B, T, CO), dtype=np.float32)
    for m in range(N_CORES):
        out[:, m * TS:(m + 1) * TS, :] = res.results[m]["out"]
    if _trace:
        kernel.last_result = res
    return out
